# revision 51
# baseline (speedup 1.0000x reference)
"""Multi-head attention (QKV projection + softmax attention) on 8 TRN2 NeuronCores.

Reference computation (per full input):
    x: [2, 8, 4, 256, 768] fp32, H=12 heads, head_dim=64
    q = split_heads(x @ Wq.T + bq); k, v likewise
    out = softmax(q k^T / sqrt(64)) v, heads merged back -> [2, 8, 4, 256, 768]

Sharding: data-parallel over the 2*8*4 = 64 independent (b,t,l) sequences,
8 sequences per core; weights replicated.

Per-core kernel design (all matmuls bf16 inputs, fp32 PSUM accumulate):
  - x and W are cast fp32->bf16 *during* the SWDGE DMA straight into SBUF
    (native layout), then transposed on the TensorE (128x128 PE-transposes
    through PSUM, drained by ScalarE copies) into contraction-major XT/WT.
    Keeping the prologue off the xbar DMA-transpose path matters: Tile cycles
    all DMAs through 8 shared semaphore lanes, and a transpose-heavy DMA
    prologue serializes on false cross-DMA lane dependencies.
  - qT, kT computed feature-major ([e, tok]) so the q k^T matmul reads them
    directly; v computed token-major ([tok, e]) so attn @ v reads it directly.
  - v bias is added via a K=1 matmul row (ones lhsT, bv rhs) which is exact
    because softmax rows sum to one; q/k biases via per-partition
    tensor_scalar adds.
  - softmax skips the max-subtraction (logits are ~N(0,1); exp is safe) and
    gets its denominator for free from a ones-column appended to v, so
    normalization is one reciprocal + one broadcast multiply per head group.
  - dots matmuls are K=64; heads are processed in (even, odd) pairs whose
    operands live at partition offsets 0/64, so the two matmuls run
    concurrently in disjoint PE row groups.
  - engines execute their instruction streams in order, so emission order is
    the schedule: attention stages (dots+exp, attn@v one pair behind, then
    normalize) are interleaved piece-by-piece with the next chunk's
    projection/transpose matmuls so the PE never sits behind a ScalarE exp.
"""

import sys

for _p in ("/opt/trn_rl_repo",):
    if _p not in sys.path:
        sys.path.insert(0, _p)

import numpy as np

import concourse.bass as bass
import concourse.tile as tile
from concourse import bacc, mybir
from concourse.bass_utils import run_bass_kernel_spmd
from concourse.masks import make_identity

N_CORES = 8
B, T_, L, P_, D = 2, 8, 4, 256, 768
H = 12
HD = D // H          # 64
NSEQ = (B * T_ * L) // N_CORES   # 8 sequences per core
NT = NSEQ * P_       # 2048 tokens per core
CT = D // 128        # 6 contraction tiles
ET = D // 128        # 6 output-feature tiles
NCHUNK = 512         # tokens per pipeline chunk
NPIPE = NT // NCHUNK # 4 chunks
SCALE = float(HD) ** -0.5
HG = 6               # heads per PSUM attn-output group

F32 = mybir.dt.float32
BF16 = mybir.dt.bfloat16


def build_nc():
    nc = bacc.Bacc()

    x = nc.dram_tensor("x", [NT, D], F32, kind="ExternalInput")
    Wq = nc.dram_tensor("Wq", [D, D], F32, kind="ExternalInput")
    Wk = nc.dram_tensor("Wk", [D, D], F32, kind="ExternalInput")
    Wv = nc.dram_tensor("Wv", [D, D], F32, kind="ExternalInput")
    bq = nc.dram_tensor("bq", [D], F32, kind="ExternalInput")
    bk = nc.dram_tensor("bk", [D], F32, kind="ExternalInput")
    bv = nc.dram_tensor("bv", [D], F32, kind="ExternalInput")
    out = nc.dram_tensor("out", [NT, D], F32, kind="ExternalOutput")

    w_in = {"q": Wq, "k": Wk, "v": Wv}

    with tile.TileContext(nc) as tc:
        with (
            tc.tile_pool(name="const", bufs=1) as const,
            tc.tile_pool(name="big", bufs=1) as big,
            tc.tile_pool(name="attn", bufs=10) as attn_pool,
            tc.tile_pool(name="rec", bufs=8) as rec_pool,
            tc.tile_pool(name="outp", bufs=6) as out_pool,
            tc.tile_pool(name="xn", bufs=2) as xn_pool,
            tc.tile_pool(name="wn", bufs=1) as wn_pool,
            tc.tile_pool(name="ps_proj", bufs=4, space="PSUM") as ps_proj,
            tc.tile_pool(name="ps_d", bufs=1, space="PSUM") as ps_d,
            tc.tile_pool(name="ps_av", bufs=2, space="PSUM") as ps_av,
        ):
            # ---- constants (biases are loaded after the hot-path DMAs) ----
            bqk = const.tile([128, 2, ET], F32)
            bvr = const.tile([1, D], BF16)
            ones = const.tile([1, 128], BF16)
            nc.vector.memset(ones[:, :], 1.0)
            ident = const.tile([128, 128], BF16)

            def emit_bias_loads():
                nc.gpsimd.dma_start(out=bqk[:, 0, :], in_=bq.rearrange("(t p) -> p t", p=128))
                nc.gpsimd.dma_start(out=bqk[:, 1, :], in_=bk.rearrange("(t p) -> p t", p=128))
                nc.gpsimd.dma_start(out=bvr[:, :], in_=bv.rearrange("(o d) -> o d", o=1))

            WT = {}
            for wk in ("q", "k", "v"):
                WT[wk] = big.tile([128, CT, D], BF16, name=f"WT_{wk}")
            XT = big.tile([128, CT, NT], BF16)     # XT[c%128, c//128, tok]

            qT = big.tile([128, ET, NT], BF16)
            kT = big.tile([128, ET, NT], BF16)
            # v_sb[tok%128, tok//128, h, 0:64] = v ; [..., 64] = 1.0
            v_sb = big.tile([128, NT // 128, H, HD + 1], BF16)
            nc.vector.memset(v_sb[:, :, :, HD:HD + 1], 1.0)

            def emit_xt_load(c):
                xbn = xn_pool.tile([128, NCHUNK // 128, D], BF16, tag="xbn",
                                   name=f"xbn_{c}")
                for hh in range(2):
                    ns = slice(c * NCHUNK + hh * (NCHUNK // 2),
                               c * NCHUNK + (hh + 1) * (NCHUNK // 2))
                    nc.gpsimd.dma_start(
                        out=xbn[:, 2 * hh:2 * hh + 2, :],
                        in_=x[ns, :].rearrange("(t p) c -> p t c", p=128),
                    )
                return xbn

            def xt_piece(c, xbn, ct):
                ns = slice(c * NCHUNK, (c + 1) * NCHUNK)
                pst = ps_proj.tile([128, NCHUNK // 128, 128], BF16, tag="ps",
                                   name=f"pst_{c}_{ct}")
                for ptl in range(NCHUNK // 128):
                    nc.tensor.transpose(
                        pst[:, ptl, :],
                        xbn[:, ptl, ct * 128:(ct + 1) * 128],
                        ident[:, :],
                    )
                nc.scalar.copy(out=XT[:, ct, ns], in_=pst[:, :, :])

            def emit_w_load(wk, split=False):
                wbn = wn_pool.tile([128, CT, D], BF16, tag="wbn",
                                   name=f"wbn_{wk}")
                nh = 2 if split else 1
                step = CT // nh
                for h0 in range(nh):
                    rs = slice(h0 * step * 128, (h0 + 1) * step * 128)
                    nc.gpsimd.dma_start(
                        out=wbn[:, h0 * step:(h0 + 1) * step, :],
                        in_=w_in[wk][rs, :].rearrange("(t p) c -> p t c", p=128),
                    )
                return wbn

            def wt_piece(wk, wbn, ct, half):
                tes = range(3 * half, 3 * half + 3)
                pst = ps_proj.tile([128, 3, 128], BF16, tag="ps",
                                   name=f"pwt_{wk}_{ct}_{half}")
                for k, te in enumerate(tes):
                    nc.tensor.transpose(
                        pst[:, k, :],
                        wbn[:, te, ct * 128:(ct + 1) * 128],
                        ident[:, :],
                    )
                nc.scalar.copy(
                    out=WT[wk][:, ct, 384 * half:384 * (half + 1)],
                    in_=pst[:, :, :],
                )

            def qk_piece(c, wk, et):
                ns = slice(c * NCHUNK, (c + 1) * NCHUNK)
                dst, brow = (qT, 0) if wk == "q" else (kT, 1)
                ps = ps_proj.tile([128, NCHUNK], F32, tag="ps", name="ps_qk")
                for ct in range(CT):
                    nc.tensor.matmul(
                        ps[:, :],
                        lhsT=WT[wk][:, ct, et * 128:(et + 1) * 128],
                        rhs=XT[:, ct, ns],
                        start=(ct == 0),
                        stop=(ct == CT - 1),
                    )
                nc.vector.tensor_scalar_add(
                    dst[:, et, ns], ps[:, :], bqk[:, brow, et:et + 1]
                )

            def v_piece(c, pt):
                for e0, ew in ((0, 512), (512, 256)):
                    ps = ps_proj.tile([128, 512], F32, tag="ps", name="ps_v")
                    for ct in range(CT):
                        nc.tensor.matmul(
                            ps[:, :ew],
                            lhsT=XT[:, ct, pt * 128:(pt + 1) * 128],
                            rhs=WT["v"][:, ct, e0:e0 + ew],
                            start=(ct == 0),
                            stop=False,
                        )
                    nc.tensor.matmul(
                        ps[:, :ew],
                        lhsT=ones[:, :],
                        rhs=bvr[:, e0:e0 + ew],
                        start=False,
                        stop=True,
                    )
                    nc.vector.tensor_copy(
                        out=v_sb[:, pt, e0 // HD:(e0 + ew) // HD, 0:HD],
                        in_=ps[:, :ew].rearrange("p (h d) -> p h d", d=HD),
                    )

            # ---- attention, in stages so projections can be interleaved ----
            class SeqAttn:
                def __init__(self, s):
                    self.s = s
                    self.tok0 = s * P_
                    self.os = [
                        out_pool.tile([128, D], F32, tag="os", name=f"os_{s}_{i}")
                        for i in range(2)
                    ]
                    self.ats = {}
                    self.pav = {}

                def dots_exp(self, jp):       # stage 1 for head pair jp
                    s, tok0 = self.s, self.tok0
                    et_h = jp  # = (2*jp)//2
                    psd = ps_d.tile([128, 2, 2, 256], F32, tag="ps_d",
                                    name=f"psd_{s}_{jp}")
                    for qt in range(2):
                        for i in range(2):
                            off = i * HD
                            nc.tensor.matmul(
                                psd[:, i, qt, :],
                                lhsT=kT[off:off + HD, et_h,
                                        tok0 + qt * 128:tok0 + (qt + 1) * 128],
                                rhs=qT[off:off + HD, et_h, tok0:tok0 + P_],
                                start=True,
                                stop=True,
                            )
                    at = attn_pool.tile([128, 2, 2, 256], BF16, tag="at",
                                        name=f"at_{s}_{jp}")
                    nc.scalar.activation(
                        out=at[:, :, :, :],
                        in_=psd[:, :, :, :],
                        func=mybir.ActivationFunctionType.Exp,
                        scale=SCALE,
                    )
                    self.ats[2 * jp] = at[:, 0]
                    self.ats[2 * jp + 1] = at[:, 1]

                def av(self, jp):             # stage 2 for head pair jp
                    s = self.s
                    g = jp // (HG // 2)
                    if g not in self.pav:
                        self.pav[g] = [
                            ps_av.tile([128, HG, HD + 1], F32, tag="ps_av",
                                       name=f"pav_{s}_{g}_{i}")
                            for i in range(2)
                        ]
                    for i in range(2):
                        h = 2 * jp + i
                        j = h - g * HG
                        at = self.ats.pop(h)
                        for pt2 in range(2):
                            for qt in range(2):
                                nc.tensor.matmul(
                                    self.pav[g][pt2][:, j, :],
                                    lhsT=at[:, qt, pt2 * 128:(pt2 + 1) * 128],
                                    rhs=v_sb[:, s * 2 + qt, h, :],
                                    start=(qt == 0),
                                    stop=(qt == 1),
                                )

                def norm(self, g):            # stage 3 for head group g
                    s = self.s
                    pav = self.pav.pop(g)
                    for pt2 in range(2):
                        rec = rec_pool.tile([128, HG, 1], F32, tag="rec",
                                            name=f"rec_{s}_{g}_{pt2}")
                        nc.vector.reciprocal(rec[:, :, :], pav[pt2][:, :, HD:HD + 1])
                        rec_b = bass.AP(
                            tensor=rec.tensor,
                            offset=rec.offset,
                            ap=[rec.ap[0], rec.ap[1], [0, HD]],
                        )
                        nc.vector.tensor_mul(
                            self.os[pt2][:, g * HG * HD:(g + 1) * HG * HD]
                            .rearrange("p (h d) -> p h d", d=HD),
                            pav[pt2][:, :, 0:HD],
                            rec_b,
                        )

                def store(self):
                    for pt2 in range(2):
                        r0 = (self.s * 2 + pt2) * 128
                        nc.sync.dma_start(
                            out=out[r0:r0 + 128, :], in_=self.os[pt2][:, :]
                        )

            def attn_slots(s):
                """Per-sequence attention as (is_dots, closure) slots in
                pipelined stage order: av runs one pair behind its dots+exp.
                is_dots marks slots after which the PE will wait on ScalarE's
                exp (PSUM slot recycling), i.e. where filler work belongs."""
                sa = SeqAttn(s)
                NP = H // 2  # 6 pairs
                slots = [(True, lambda sa=sa, j=0: sa.dots_exp(j))]
                for j in range(1, NP):
                    slots.append((True, lambda sa=sa, j=j: sa.dots_exp(j)))
                    slots.append((False, lambda sa=sa, j=j - 1: sa.av(j)))
                    if j - 1 == HG // 2 - 1:
                        slots.append((False, lambda sa=sa: sa.norm(0)))
                slots.append((False, lambda sa=sa, j=NP - 1: sa.av(j)))
                slots.append((False, lambda sa=sa: (sa.norm(1), sa.store())))
                return slots

            # ---- prologue ----
            xn0 = emit_xt_load(0)
            wq_nat = emit_w_load("q", split=True)
            # dependency-free warm-up matmuls: the PE would otherwise idle
            # until the first x/W DMA lands (~14us), and the HAM clock gate
            # needs ~3.4us of sustained activity to lift the PE from 1.2 to
            # 2.4 GHz. Warming during the DMA wait makes the real prologue
            # transposes/projections run at full clock.
            warm = const.tile([128, 512], BF16)
            nc.vector.memset(warm[:, :], 0.0)
            wps = ps_proj.tile([128, 512], F32, tag="ps", name="warm_ps")
            for _ in range(16):
                nc.tensor.matmul(
                    wps[:, :], lhsT=warm[:, 0:128], rhs=warm[:, :],
                    start=True, stop=True,
                )
            make_identity(nc, ident[:, :])
            emit_bias_loads()
            for ct in range(CT):
                xt_piece(0, xn0, ct)
            for ct in range(CT):
                for half in range(2):
                    wt_piece("q", wq_nat, ct, half)
            for et in range(ET):
                qk_piece(0, "q", et)
            wk_nat = emit_w_load("k", split=True)
            for ct in range(CT):
                for half in range(2):
                    wt_piece("k", wk_nat, ct, half)
            for et in range(ET):
                qk_piece(0, "k", et)
            wv_nat = emit_w_load("v", split=True)
            for ct in range(CT):
                for half in range(2):
                    wt_piece("v", wv_nat, ct, half)
            for pt in range(4):
                v_piece(0, pt)

            # ---- steady state: interleave attn(c) with chunk c+1's work ----
            for c in range(NPIPE):
                bq_pieces = []
                if c + 1 < NPIPE:
                    xn = emit_xt_load(c + 1)
                    bq_pieces += [
                        (lambda cc=c + 1, xb=xn, ct=ct: xt_piece(cc, xb, ct))
                        for ct in range(CT)
                    ]
                    for et in range(ET):
                        bq_pieces.append(lambda cc=c + 1, et=et: qk_piece(cc, "q", et))
                        bq_pieces.append(lambda cc=c + 1, et=et: qk_piece(cc, "k", et))
                    bq_pieces += [
                        (lambda cc=c + 1, pt=pt: v_piece(cc, pt))
                        for pt in range((c + 1) * 4, (c + 2) * 4)
                    ]
                a_slots = []
                for sloc in range(NCHUNK // P_):
                    a_slots += attn_slots(c * (NCHUNK // P_) + sloc)
                # round-robin merge: spread B pieces evenly between A slots
                nb, na = len(bq_pieces), len(a_slots)
                bi = 0
                for ai, (is_dots, slot) in enumerate(a_slots):
                    slot()
                    # keep the even spread, but guarantee one filler right
                    # after every dots+exp slot where the PE stalls next
                    want = ((ai + 1) * nb) // na
                    if is_dots:
                        want = max(want, bi + 1)
                    want = min(want, nb)
                    while bi < want:
                        bq_pieces[bi]()
                        bi += 1
                while bi < nb:
                    bq_pieces[bi]()
                    bi += 1

    nc.finalize()
    return nc


_NC_CACHE = {}


def _get_nc():
    if "nc" not in _NC_CACHE:
        _NC_CACHE["nc"] = build_nc()
    return _NC_CACHE["nc"]


def kernel(x, Wq, bq, Wk, bk, Wv, bv):
    x = np.ascontiguousarray(np.asarray(x, dtype=np.float32))
    args = {
        "Wq": np.ascontiguousarray(np.asarray(Wq, dtype=np.float32)),
        "Wk": np.ascontiguousarray(np.asarray(Wk, dtype=np.float32)),
        "Wv": np.ascontiguousarray(np.asarray(Wv, dtype=np.float32)),
        "bq": np.ascontiguousarray(np.asarray(bq, dtype=np.float32)),
        "bk": np.ascontiguousarray(np.asarray(bk, dtype=np.float32)),
        "bv": np.ascontiguousarray(np.asarray(bv, dtype=np.float32)),
    }
    xf = x.reshape(B * T_ * L * P_, D)
    nc = _get_nc()
    in_maps = [
        {"x": xf[i * NT:(i + 1) * NT], **args} for i in range(N_CORES)
    ]
    res = run_bass_kernel_spmd(nc, in_maps, list(range(N_CORES)))
    outs = [res.results[i]["out"] for i in range(N_CORES)]
    full = np.concatenate(outs, axis=0).reshape(B, T_, L, P_, D)
    return full.astype(np.float32)


# revision 52
# speedup vs baseline: 1.0370x; 1.0370x over previous
"""Multi-head attention (QKV projection + softmax attention) on 8 TRN2 NeuronCores.

Reference computation (per full input):
    x: [2, 8, 4, 256, 768] fp32, H=12 heads, head_dim=64
    q = split_heads(x @ Wq.T + bq); k, v likewise
    out = softmax(q k^T / sqrt(64)) v, heads merged back -> [2, 8, 4, 256, 768]

Sharding: data-parallel over the 2*8*4 = 64 independent (b,t,l) sequences,
8 sequences per core; weights replicated.

Per-core kernel design (all matmuls bf16 inputs, fp32 PSUM accumulate):
  - x and W are cast fp32->bf16 *during* the SWDGE DMA straight into SBUF
    (native layout), then transposed on the TensorE (128x128 PE-transposes
    through PSUM, drained by ScalarE copies) into contraction-major XT/WT.
    Keeping the prologue off the xbar DMA-transpose path matters: Tile cycles
    all DMAs through 8 shared semaphore lanes, and a transpose-heavy DMA
    prologue serializes on false cross-DMA lane dependencies.
  - qT, kT computed feature-major ([e, tok]) so the q k^T matmul reads them
    directly; v computed token-major ([tok, e]) so attn @ v reads it directly.
  - v bias is added via a K=1 matmul row (ones lhsT, bv rhs) which is exact
    because softmax rows sum to one; q/k biases via per-partition
    tensor_scalar adds.
  - softmax skips the max-subtraction (logits are ~N(0,1); exp is safe) and
    gets its denominator for free from a ones-column appended to v, so
    normalization is one reciprocal + one broadcast multiply per head group.
  - dots matmuls are K=64; heads are processed in (even, odd) pairs whose
    operands live at partition offsets 0/64, so the two matmuls run
    concurrently in disjoint PE row groups.
  - engines execute their instruction streams in order, so emission order is
    the schedule: attention stages (dots+exp, attn@v one pair behind, then
    normalize) are interleaved piece-by-piece with the next chunk's
    projection/transpose matmuls so the PE never sits behind a ScalarE exp.
"""

import sys

for _p in ("/opt/trn_rl_repo",):
    if _p not in sys.path:
        sys.path.insert(0, _p)

import numpy as np

import concourse.bass as bass
import concourse.tile as tile
from concourse import bacc, mybir
from concourse.bass_utils import run_bass_kernel_spmd
from concourse.masks import make_identity

N_CORES = 8
B, T_, L, P_, D = 2, 8, 4, 256, 768
H = 12
HD = D // H          # 64
NSEQ = (B * T_ * L) // N_CORES   # 8 sequences per core
NT = NSEQ * P_       # 2048 tokens per core
CT = D // 128        # 6 contraction tiles
ET = D // 128        # 6 output-feature tiles
NCHUNK = 512         # tokens per pipeline chunk
NPIPE = NT // NCHUNK # 4 chunks
SCALE = float(HD) ** -0.5
HG = 6               # heads per PSUM attn-output group

F32 = mybir.dt.float32
BF16 = mybir.dt.bfloat16


def build_nc():
    nc = bacc.Bacc()

    x = nc.dram_tensor("x", [NT, D], F32, kind="ExternalInput")
    Wq = nc.dram_tensor("Wq", [D, D], F32, kind="ExternalInput")
    Wk = nc.dram_tensor("Wk", [D, D], F32, kind="ExternalInput")
    Wv = nc.dram_tensor("Wv", [D, D], F32, kind="ExternalInput")
    bq = nc.dram_tensor("bq", [D], F32, kind="ExternalInput")
    bk = nc.dram_tensor("bk", [D], F32, kind="ExternalInput")
    bv = nc.dram_tensor("bv", [D], F32, kind="ExternalInput")
    out = nc.dram_tensor("out", [NT, D], F32, kind="ExternalOutput")

    w_in = {"q": Wq, "k": Wk, "v": Wv}

    with tile.TileContext(nc) as tc:
        with (
            tc.tile_pool(name="const", bufs=1) as const,
            tc.tile_pool(name="big", bufs=1) as big,
            tc.tile_pool(name="attn", bufs=10) as attn_pool,
            tc.tile_pool(name="rec", bufs=8) as rec_pool,
            tc.tile_pool(name="outp", bufs=6) as out_pool,
            tc.tile_pool(name="xn", bufs=2) as xn_pool,
            tc.tile_pool(name="wn", bufs=1) as wn_pool,
            tc.tile_pool(name="ps_proj", bufs=4, space="PSUM") as ps_proj,
            tc.tile_pool(name="ps_d", bufs=1, space="PSUM") as ps_d,
            tc.tile_pool(name="ps_av", bufs=2, space="PSUM") as ps_av,
        ):
            # ---- constants (biases are loaded after the hot-path DMAs) ----
            bqk = const.tile([128, 2, ET], F32)
            bvr = const.tile([1, D], BF16)
            ones = const.tile([1, 128], BF16)
            nc.vector.memset(ones[:, :], 1.0)
            ident = const.tile([128, 128], BF16)

            def emit_bias_loads():
                nc.gpsimd.dma_start(out=bqk[:, 0, :], in_=bq.rearrange("(t p) -> p t", p=128))
                nc.gpsimd.dma_start(out=bqk[:, 1, :], in_=bk.rearrange("(t p) -> p t", p=128))
                nc.gpsimd.dma_start(out=bvr[:, :], in_=bv.rearrange("(o d) -> o d", o=1))

            WT = {}
            for wk in ("q", "k", "v"):
                WT[wk] = big.tile([128, CT, D], BF16, name=f"WT_{wk}")
            XT = big.tile([128, CT, NT], BF16)     # XT[c%128, c//128, tok]

            qT = big.tile([128, ET, NT], BF16)
            kT = big.tile([128, ET, NT], BF16)
            # v_sb[tok%128, tok//128, h, 0:64] = v ; [..., 64] = 1.0
            v_sb = big.tile([128, NT // 128, H, HD + 1], BF16)
            nc.vector.memset(v_sb[:, :, :, HD:HD + 1], 1.0)

            def emit_xt_load(c):
                xbn = xn_pool.tile([128, NCHUNK // 128, D], BF16, tag="xbn",
                                   name=f"xbn_{c}")
                for hh in range(2):
                    ns = slice(c * NCHUNK + hh * (NCHUNK // 2),
                               c * NCHUNK + (hh + 1) * (NCHUNK // 2))
                    nc.gpsimd.dma_start(
                        out=xbn[:, 2 * hh:2 * hh + 2, :],
                        in_=x[ns, :].rearrange("(t p) c -> p t c", p=128),
                    )
                return xbn

            def xt_piece(c, xbn, ct):
                ns = slice(c * NCHUNK, (c + 1) * NCHUNK)
                pst = ps_proj.tile([128, NCHUNK // 128, 128], BF16, tag="ps",
                                   name=f"pst_{c}_{ct}")
                for ptl in range(NCHUNK // 128):
                    nc.tensor.transpose(
                        pst[:, ptl, :],
                        xbn[:, ptl, ct * 128:(ct + 1) * 128],
                        ident[:, :],
                    )
                nc.scalar.copy(out=XT[:, ct, ns], in_=pst[:, :, :])

            def emit_w_load(wk, split=False):
                wbn = wn_pool.tile([128, CT, D], BF16, tag="wbn",
                                   name=f"wbn_{wk}")
                nh = 2 if split else 1
                step = CT // nh
                for h0 in range(nh):
                    rs = slice(h0 * step * 128, (h0 + 1) * step * 128)
                    nc.gpsimd.dma_start(
                        out=wbn[:, h0 * step:(h0 + 1) * step, :],
                        in_=w_in[wk][rs, :].rearrange("(t p) c -> p t c", p=128),
                    )
                return wbn

            def wt_piece(wk, wbn, ct, half):
                tes = range(3 * half, 3 * half + 3)
                pst = ps_proj.tile([128, 3, 128], BF16, tag="ps",
                                   name=f"pwt_{wk}_{ct}_{half}")
                for k, te in enumerate(tes):
                    nc.tensor.transpose(
                        pst[:, k, :],
                        wbn[:, te, ct * 128:(ct + 1) * 128],
                        ident[:, :],
                    )
                nc.scalar.copy(
                    out=WT[wk][:, ct, 384 * half:384 * (half + 1)],
                    in_=pst[:, :, :],
                )

            def qk_piece(c, wk, et):
                ns = slice(c * NCHUNK, (c + 1) * NCHUNK)
                dst, brow = (qT, 0) if wk == "q" else (kT, 1)
                ps = ps_proj.tile([128, NCHUNK], F32, tag="ps", name="ps_qk")
                for ct in range(CT):
                    nc.tensor.matmul(
                        ps[:, :],
                        lhsT=WT[wk][:, ct, et * 128:(et + 1) * 128],
                        rhs=XT[:, ct, ns],
                        start=(ct == 0),
                        stop=(ct == CT - 1),
                    )
                nc.vector.tensor_scalar_add(
                    dst[:, et, ns], ps[:, :], bqk[:, brow, et:et + 1]
                )

            def v_piece(c, pt):
                for e0, ew in ((0, 512), (512, 256)):
                    ps = ps_proj.tile([128, 512], F32, tag="ps", name="ps_v")
                    for ct in range(CT):
                        nc.tensor.matmul(
                            ps[:, :ew],
                            lhsT=XT[:, ct, pt * 128:(pt + 1) * 128],
                            rhs=WT["v"][:, ct, e0:e0 + ew],
                            start=(ct == 0),
                            stop=False,
                        )
                    nc.tensor.matmul(
                        ps[:, :ew],
                        lhsT=ones[:, :],
                        rhs=bvr[:, e0:e0 + ew],
                        start=False,
                        stop=True,
                    )
                    nc.vector.tensor_copy(
                        out=v_sb[:, pt, e0 // HD:(e0 + ew) // HD, 0:HD],
                        in_=ps[:, :ew].rearrange("p (h d) -> p h d", d=HD),
                    )

            # ---- attention, in stages so projections can be interleaved ----
            class SeqAttn:
                def __init__(self, s):
                    self.s = s
                    self.tok0 = s * P_
                    self.os = [
                        out_pool.tile([128, D], F32, tag="os", name=f"os_{s}_{i}")
                        for i in range(2)
                    ]
                    self.ats = {}
                    self.pav = {}

                def dots_exp(self, jp, tail=False):   # stage 1 for head pair jp
                    s, tok0 = self.s, self.tok0
                    et_h = jp  # = (2*jp)//2
                    at = attn_pool.tile([128, 2, 2, 256], BF16, tag="at",
                                        name=f"at_{s}_{jp}")
                    if tail:
                        # final period: no projection work remains, so borrow
                        # the (otherwise idle) 4-deep projection PSUM pool for
                        # per-head dots tiles; with 4 slots in flight the next
                        # pair's dots no longer wait on this pair's exp.
                        for i in range(2):
                            off = i * HD
                            psd = ps_proj.tile([128, 2, 256], F32, tag="ps",
                                               name=f"psdt_{s}_{jp}_{i}")
                            for qt in range(2):
                                nc.tensor.matmul(
                                    psd[:, qt, :],
                                    lhsT=kT[off:off + HD, et_h,
                                            tok0 + qt * 128:tok0 + (qt + 1) * 128],
                                    rhs=qT[off:off + HD, et_h, tok0:tok0 + P_],
                                    start=True,
                                    stop=True,
                                )
                            nc.scalar.activation(
                                out=at[:, i, :, :],
                                in_=psd[:, :, :],
                                func=mybir.ActivationFunctionType.Exp,
                                scale=SCALE,
                            )
                    else:
                        psd = ps_d.tile([128, 2, 2, 256], F32, tag="ps_d",
                                        name=f"psd_{s}_{jp}")
                        for qt in range(2):
                            for i in range(2):
                                off = i * HD
                                nc.tensor.matmul(
                                    psd[:, i, qt, :],
                                    lhsT=kT[off:off + HD, et_h,
                                            tok0 + qt * 128:tok0 + (qt + 1) * 128],
                                    rhs=qT[off:off + HD, et_h, tok0:tok0 + P_],
                                    start=True,
                                    stop=True,
                                )
                        nc.scalar.activation(
                            out=at[:, :, :, :],
                            in_=psd[:, :, :, :],
                            func=mybir.ActivationFunctionType.Exp,
                            scale=SCALE,
                        )
                    self.ats[2 * jp] = at[:, 0]
                    self.ats[2 * jp + 1] = at[:, 1]

                def av(self, jp):             # stage 2 for head pair jp
                    s = self.s
                    g = jp // (HG // 2)
                    if g not in self.pav:
                        self.pav[g] = [
                            ps_av.tile([128, HG, HD + 1], F32, tag="ps_av",
                                       name=f"pav_{s}_{g}_{i}")
                            for i in range(2)
                        ]
                    for i in range(2):
                        h = 2 * jp + i
                        j = h - g * HG
                        at = self.ats.pop(h)
                        for pt2 in range(2):
                            for qt in range(2):
                                nc.tensor.matmul(
                                    self.pav[g][pt2][:, j, :],
                                    lhsT=at[:, qt, pt2 * 128:(pt2 + 1) * 128],
                                    rhs=v_sb[:, s * 2 + qt, h, :],
                                    start=(qt == 0),
                                    stop=(qt == 1),
                                )

                def norm(self, g):            # stage 3 for head group g
                    s = self.s
                    pav = self.pav.pop(g)
                    for pt2 in range(2):
                        rec = rec_pool.tile([128, HG, 1], F32, tag="rec",
                                            name=f"rec_{s}_{g}_{pt2}")
                        nc.vector.reciprocal(rec[:, :, :], pav[pt2][:, :, HD:HD + 1])
                        rec_b = bass.AP(
                            tensor=rec.tensor,
                            offset=rec.offset,
                            ap=[rec.ap[0], rec.ap[1], [0, HD]],
                        )
                        nc.vector.tensor_mul(
                            self.os[pt2][:, g * HG * HD:(g + 1) * HG * HD]
                            .rearrange("p (h d) -> p h d", d=HD),
                            pav[pt2][:, :, 0:HD],
                            rec_b,
                        )

                def store(self):
                    for pt2 in range(2):
                        r0 = (self.s * 2 + pt2) * 128
                        nc.sync.dma_start(
                            out=out[r0:r0 + 128, :], in_=self.os[pt2][:, :]
                        )

            def attn_slots(s, tail=False):
                """Per-sequence attention as (is_dots, closure) slots in
                pipelined stage order: av runs one pair behind its dots+exp.
                is_dots marks slots after which the PE will wait on ScalarE's
                exp (PSUM slot recycling), i.e. where filler work belongs."""
                sa = SeqAttn(s)
                NP = H // 2  # 6 pairs
                slots = [(True, lambda sa=sa, j=0: sa.dots_exp(j, tail))]
                for j in range(1, NP):
                    slots.append((True, lambda sa=sa, j=j: sa.dots_exp(j, tail)))
                    slots.append((False, lambda sa=sa, j=j - 1: sa.av(j)))
                    if j - 1 == HG // 2 - 1:
                        slots.append((False, lambda sa=sa: sa.norm(0)))
                slots.append((False, lambda sa=sa, j=NP - 1: sa.av(j)))
                slots.append((False, lambda sa=sa: (sa.norm(1), sa.store())))
                return slots

            # ---- prologue ----
            xn0 = emit_xt_load(0)
            wq_nat = emit_w_load("q", split=True)
            # dependency-free warm-up matmuls: the PE would otherwise idle
            # until the first x/W DMA lands (~14us), and the HAM clock gate
            # needs ~3.4us of sustained activity to lift the PE from 1.2 to
            # 2.4 GHz. Warming during the DMA wait makes the real prologue
            # transposes/projections run at full clock.
            warm = const.tile([128, 512], BF16)
            nc.vector.memset(warm[:, :], 0.0)
            wps = ps_proj.tile([128, 512], F32, tag="ps", name="warm_ps")
            for _ in range(16):
                nc.tensor.matmul(
                    wps[:, :], lhsT=warm[:, 0:128], rhs=warm[:, :],
                    start=True, stop=True,
                )
            make_identity(nc, ident[:, :])
            emit_bias_loads()
            for ct in range(CT):
                xt_piece(0, xn0, ct)
            for ct in range(CT):
                for half in range(2):
                    wt_piece("q", wq_nat, ct, half)
            for et in range(ET):
                qk_piece(0, "q", et)
            wk_nat = emit_w_load("k", split=True)
            for ct in range(CT):
                for half in range(2):
                    wt_piece("k", wk_nat, ct, half)
            for et in range(ET):
                qk_piece(0, "k", et)
            wv_nat = emit_w_load("v", split=True)
            for ct in range(CT):
                for half in range(2):
                    wt_piece("v", wv_nat, ct, half)
            for pt in range(4):
                v_piece(0, pt)

            # ---- steady state: interleave attn(c) with chunk c+1's work ----
            for c in range(NPIPE):
                bq_pieces = []
                if c + 1 < NPIPE:
                    xn = emit_xt_load(c + 1)
                    bq_pieces += [
                        (lambda cc=c + 1, xb=xn, ct=ct: xt_piece(cc, xb, ct))
                        for ct in range(CT)
                    ]
                    for et in range(ET):
                        bq_pieces.append(lambda cc=c + 1, et=et: qk_piece(cc, "q", et))
                        bq_pieces.append(lambda cc=c + 1, et=et: qk_piece(cc, "k", et))
                    bq_pieces += [
                        (lambda cc=c + 1, pt=pt: v_piece(cc, pt))
                        for pt in range((c + 1) * 4, (c + 2) * 4)
                    ]
                a_slots = []
                for sloc in range(NCHUNK // P_):
                    a_slots += attn_slots(c * (NCHUNK // P_) + sloc,
                                          tail=(c + 1 == NPIPE))
                # round-robin merge: spread B pieces evenly between A slots
                nb, na = len(bq_pieces), len(a_slots)
                bi = 0
                for ai, (is_dots, slot) in enumerate(a_slots):
                    slot()
                    # keep the even spread, but guarantee one filler right
                    # after every dots+exp slot where the PE stalls next
                    want = ((ai + 1) * nb) // na
                    if is_dots:
                        want = max(want, bi + 1)
                    want = min(want, nb)
                    while bi < want:
                        bq_pieces[bi]()
                        bi += 1
                while bi < nb:
                    bq_pieces[bi]()
                    bi += 1

    nc.finalize()
    return nc


_NC_CACHE = {}


def _get_nc():
    if "nc" not in _NC_CACHE:
        _NC_CACHE["nc"] = build_nc()
    return _NC_CACHE["nc"]


def kernel(x, Wq, bq, Wk, bk, Wv, bv):
    x = np.ascontiguousarray(np.asarray(x, dtype=np.float32))
    args = {
        "Wq": np.ascontiguousarray(np.asarray(Wq, dtype=np.float32)),
        "Wk": np.ascontiguousarray(np.asarray(Wk, dtype=np.float32)),
        "Wv": np.ascontiguousarray(np.asarray(Wv, dtype=np.float32)),
        "bq": np.ascontiguousarray(np.asarray(bq, dtype=np.float32)),
        "bk": np.ascontiguousarray(np.asarray(bk, dtype=np.float32)),
        "bv": np.ascontiguousarray(np.asarray(bv, dtype=np.float32)),
    }
    xf = x.reshape(B * T_ * L * P_, D)
    nc = _get_nc()
    in_maps = [
        {"x": xf[i * NT:(i + 1) * NT], **args} for i in range(N_CORES)
    ]
    res = run_bass_kernel_spmd(nc, in_maps, list(range(N_CORES)))
    outs = [res.results[i]["out"] for i in range(N_CORES)]
    full = np.concatenate(outs, axis=0).reshape(B, T_, L, P_, D)
    return full.astype(np.float32)


# revision 54
# speedup vs baseline: 1.0413x; 1.0041x over previous
"""Multi-head attention (QKV projection + softmax attention) on 8 TRN2 NeuronCores.

Reference computation (per full input):
    x: [2, 8, 4, 256, 768] fp32, H=12 heads, head_dim=64
    q = split_heads(x @ Wq.T + bq); k, v likewise
    out = softmax(q k^T / sqrt(64)) v, heads merged back -> [2, 8, 4, 256, 768]

Sharding: data-parallel over the 2*8*4 = 64 independent (b,t,l) sequences,
8 sequences per core; weights replicated.

Per-core kernel design (all matmuls bf16 inputs, fp32 PSUM accumulate):
  - x and W are cast fp32->bf16 *during* the SWDGE DMA straight into SBUF
    (native layout), then transposed on the TensorE (128x128 PE-transposes
    through PSUM, drained by ScalarE copies) into contraction-major XT/WT.
    Keeping the prologue off the xbar DMA-transpose path matters: Tile cycles
    all DMAs through 8 shared semaphore lanes, and a transpose-heavy DMA
    prologue serializes on false cross-DMA lane dependencies.
  - qT, kT computed feature-major ([e, tok]) so the q k^T matmul reads them
    directly; v computed token-major ([tok, e]) so attn @ v reads it directly.
  - v bias is added via a K=1 matmul row (ones lhsT, bv rhs) which is exact
    because softmax rows sum to one; q/k biases via per-partition
    tensor_scalar adds.
  - softmax skips the max-subtraction (logits are ~N(0,1); exp is safe) and
    gets its denominator for free from a ones-column appended to v, so
    normalization is one reciprocal + one broadcast multiply per head group.
  - dots matmuls are K=64; heads are processed in (even, odd) pairs whose
    operands live at partition offsets 0/64, so the two matmuls run
    concurrently in disjoint PE row groups.
  - engines execute their instruction streams in order, so emission order is
    the schedule: attention stages (dots+exp, attn@v one pair behind, then
    normalize) are interleaved piece-by-piece with the next chunk's
    projection/transpose matmuls so the PE never sits behind a ScalarE exp.
"""

import sys

for _p in ("/opt/trn_rl_repo",):
    if _p not in sys.path:
        sys.path.insert(0, _p)

import numpy as np

import concourse.bass as bass
import concourse.tile as tile
from concourse import bacc, mybir
from concourse.bass_utils import run_bass_kernel_spmd
from concourse.masks import make_identity

N_CORES = 8
B, T_, L, P_, D = 2, 8, 4, 256, 768
H = 12
HD = D // H          # 64
NSEQ = (B * T_ * L) // N_CORES   # 8 sequences per core
NT = NSEQ * P_       # 2048 tokens per core
CT = D // 128        # 6 contraction tiles
ET = D // 128        # 6 output-feature tiles
NCHUNK = 512         # tokens per pipeline chunk
NPIPE = NT // NCHUNK # 4 chunks
SCALE = float(HD) ** -0.5
HG = 6               # heads per PSUM attn-output group

F32 = mybir.dt.float32
BF16 = mybir.dt.bfloat16


def build_nc():
    nc = bacc.Bacc()

    x = nc.dram_tensor("x", [NT, D], F32, kind="ExternalInput")
    Wq = nc.dram_tensor("Wq", [D, D], F32, kind="ExternalInput")
    Wk = nc.dram_tensor("Wk", [D, D], F32, kind="ExternalInput")
    Wv = nc.dram_tensor("Wv", [D, D], F32, kind="ExternalInput")
    bq = nc.dram_tensor("bq", [D], F32, kind="ExternalInput")
    bk = nc.dram_tensor("bk", [D], F32, kind="ExternalInput")
    bv = nc.dram_tensor("bv", [D], F32, kind="ExternalInput")
    out = nc.dram_tensor("out", [NT, D], F32, kind="ExternalOutput")

    w_in = {"q": Wq, "k": Wk, "v": Wv}

    with tile.TileContext(nc) as tc:
        with (
            tc.tile_pool(name="const", bufs=1) as const,
            tc.tile_pool(name="big", bufs=1) as big,
            tc.tile_pool(name="attn", bufs=10) as attn_pool,
            tc.tile_pool(name="rec", bufs=8) as rec_pool,
            tc.tile_pool(name="outp", bufs=6) as out_pool,
            tc.tile_pool(name="xn", bufs=2) as xn_pool,
            tc.tile_pool(name="wn", bufs=1) as wn_pool,
            tc.tile_pool(name="ps_proj", bufs=4, space="PSUM") as ps_proj,
            tc.tile_pool(name="ps_d", bufs=1, space="PSUM") as ps_d,
            tc.tile_pool(name="ps_av", bufs=2, space="PSUM") as ps_av,
        ):
            # ---- constants (biases are loaded after the hot-path DMAs) ----
            bqk = const.tile([128, 2, ET], F32)
            bvr = const.tile([1, D], BF16)
            ones = const.tile([1, 128], BF16)
            nc.vector.memset(ones[:, :], 1.0)
            ident = const.tile([128, 128], BF16)

            def emit_bias_loads():
                nc.gpsimd.dma_start(out=bqk[:, 0, :], in_=bq.rearrange("(t p) -> p t", p=128))
                nc.gpsimd.dma_start(out=bqk[:, 1, :], in_=bk.rearrange("(t p) -> p t", p=128))
                nc.gpsimd.dma_start(out=bvr[:, :], in_=bv.rearrange("(o d) -> o d", o=1))

            WT = {}
            for wk in ("q", "k", "v"):
                WT[wk] = big.tile([128, CT, D], BF16, name=f"WT_{wk}")
            XT = big.tile([128, CT, NT], BF16)     # XT[c%128, c//128, tok]

            qT = big.tile([128, ET, NT], BF16)
            kT = big.tile([128, ET, NT], BF16)
            # v_sb[tok%128, tok//128, h, 0:64] = v ; [..., 64] = 1.0
            v_sb = big.tile([128, NT // 128, H, HD + 1], BF16)
            nc.vector.memset(v_sb[:, :, :, HD:HD + 1], 1.0)

            def emit_xt_load(c):
                xbn = xn_pool.tile([128, NCHUNK // 128, D], BF16, tag="xbn",
                                   name=f"xbn_{c}")
                for hh in range(2):
                    ns = slice(c * NCHUNK + hh * (NCHUNK // 2),
                               c * NCHUNK + (hh + 1) * (NCHUNK // 2))
                    nc.gpsimd.dma_start(
                        out=xbn[:, 2 * hh:2 * hh + 2, :],
                        in_=x[ns, :].rearrange("(t p) c -> p t c", p=128),
                    )
                return xbn

            def xt_piece(c, xbn, ct):
                ns = slice(c * NCHUNK, (c + 1) * NCHUNK)
                pst = ps_proj.tile([128, NCHUNK // 128, 128], BF16, tag="ps",
                                   name=f"pst_{c}_{ct}")
                for ptl in range(NCHUNK // 128):
                    nc.tensor.transpose(
                        pst[:, ptl, :],
                        xbn[:, ptl, ct * 128:(ct + 1) * 128],
                        ident[:, :],
                    )
                nc.scalar.copy(out=XT[:, ct, ns], in_=pst[:, :, :])

            def emit_w_load(wk, split=False):
                wbn = wn_pool.tile([128, CT, D], BF16, tag="wbn",
                                   name=f"wbn_{wk}")
                nh = 2 if split else 1
                step = CT // nh
                for h0 in range(nh):
                    rs = slice(h0 * step * 128, (h0 + 1) * step * 128)
                    nc.gpsimd.dma_start(
                        out=wbn[:, h0 * step:(h0 + 1) * step, :],
                        in_=w_in[wk][rs, :].rearrange("(t p) c -> p t c", p=128),
                    )
                return wbn

            def wt_piece(wk, wbn, ct, half):
                tes = range(3 * half, 3 * half + 3)
                pst = ps_proj.tile([128, 3, 128], BF16, tag="ps",
                                   name=f"pwt_{wk}_{ct}_{half}")
                for k, te in enumerate(tes):
                    nc.tensor.transpose(
                        pst[:, k, :],
                        wbn[:, te, ct * 128:(ct + 1) * 128],
                        ident[:, :],
                    )
                nc.scalar.copy(
                    out=WT[wk][:, ct, 384 * half:384 * (half + 1)],
                    in_=pst[:, :, :],
                )

            def qk_piece(c, wk, et):
                ns = slice(c * NCHUNK, (c + 1) * NCHUNK)
                dst, brow = (qT, 0) if wk == "q" else (kT, 1)
                ps = ps_proj.tile([128, NCHUNK], F32, tag="ps", name="ps_qk")
                for ct in range(CT):
                    nc.tensor.matmul(
                        ps[:, :],
                        lhsT=WT[wk][:, ct, et * 128:(et + 1) * 128],
                        rhs=XT[:, ct, ns],
                        start=(ct == 0),
                        stop=(ct == CT - 1),
                    )
                nc.vector.tensor_scalar_add(
                    dst[:, et, ns], ps[:, :], bqk[:, brow, et:et + 1]
                )

            def v_piece(c, pt):
                for e0, ew in ((0, 512), (512, 256)):
                    ps = ps_proj.tile([128, 512], F32, tag="ps", name="ps_v")
                    for ct in range(CT):
                        nc.tensor.matmul(
                            ps[:, :ew],
                            lhsT=XT[:, ct, pt * 128:(pt + 1) * 128],
                            rhs=WT["v"][:, ct, e0:e0 + ew],
                            start=(ct == 0),
                            stop=False,
                        )
                    nc.tensor.matmul(
                        ps[:, :ew],
                        lhsT=ones[:, :],
                        rhs=bvr[:, e0:e0 + ew],
                        start=False,
                        stop=True,
                    )
                    nc.vector.tensor_copy(
                        out=v_sb[:, pt, e0 // HD:(e0 + ew) // HD, 0:HD],
                        in_=ps[:, :ew].rearrange("p (h d) -> p h d", d=HD),
                    )

            # ---- attention, in stages so projections can be interleaved ----
            class SeqAttn:
                def __init__(self, s):
                    self.s = s
                    self.tok0 = s * P_
                    self.os = [
                        out_pool.tile([128, D], F32, tag="os", name=f"os_{s}_{i}")
                        for i in range(2)
                    ]
                    self.ats = {}
                    self.pav = {}

                def dots_exp(self, jp, tail=False):   # stage 1 for head pair jp
                    s, tok0 = self.s, self.tok0
                    et_h = jp  # = (2*jp)//2
                    at = attn_pool.tile([128, 2, 2, 256], BF16, tag="at",
                                        name=f"at_{s}_{jp}")
                    if tail:
                        # final period: no projection work remains, so borrow
                        # the (otherwise idle) 4-deep projection PSUM pool for
                        # per-head dots tiles; with 4 slots in flight the next
                        # pair's dots no longer wait on this pair's exp.
                        for i in range(2):
                            off = i * HD
                            psd = ps_proj.tile([128, 2, 256], F32, tag="ps",
                                               name=f"psdt_{s}_{jp}_{i}")
                            for qt in range(2):
                                nc.tensor.matmul(
                                    psd[:, qt, :],
                                    lhsT=kT[off:off + HD, et_h,
                                            tok0 + qt * 128:tok0 + (qt + 1) * 128],
                                    rhs=qT[off:off + HD, et_h, tok0:tok0 + P_],
                                    start=True,
                                    stop=True,
                                )
                            nc.scalar.activation(
                                out=at[:, i, :, :],
                                in_=psd[:, :, :],
                                func=mybir.ActivationFunctionType.Exp,
                                scale=SCALE,
                            )
                    else:
                        psd = ps_d.tile([128, 2, 2, 256], F32, tag="ps_d",
                                        name=f"psd_{s}_{jp}")
                        for qt in range(2):
                            for i in range(2):
                                off = i * HD
                                nc.tensor.matmul(
                                    psd[:, i, qt, :],
                                    lhsT=kT[off:off + HD, et_h,
                                            tok0 + qt * 128:tok0 + (qt + 1) * 128],
                                    rhs=qT[off:off + HD, et_h, tok0:tok0 + P_],
                                    start=True,
                                    stop=True,
                                )
                        nc.scalar.activation(
                            out=at[:, :, :, :],
                            in_=psd[:, :, :, :],
                            func=mybir.ActivationFunctionType.Exp,
                            scale=SCALE,
                        )
                    self.ats[2 * jp] = at[:, 0]
                    self.ats[2 * jp + 1] = at[:, 1]

                def av(self, jp):             # stage 2 for head pair jp
                    s = self.s
                    g = jp // (HG // 2)
                    if g not in self.pav:
                        self.pav[g] = [
                            ps_av.tile([128, HG, HD + 1], F32, tag="ps_av",
                                       name=f"pav_{s}_{g}_{i}")
                            for i in range(2)
                        ]
                    for i in range(2):
                        h = 2 * jp + i
                        j = h - g * HG
                        at = self.ats.pop(h)
                        for pt2 in range(2):
                            for qt in range(2):
                                nc.tensor.matmul(
                                    self.pav[g][pt2][:, j, :],
                                    lhsT=at[:, qt, pt2 * 128:(pt2 + 1) * 128],
                                    rhs=v_sb[:, s * 2 + qt, h, :],
                                    start=(qt == 0),
                                    stop=(qt == 1),
                                )

                def norm(self, g):            # stage 3 for head group g
                    s = self.s
                    pav = self.pav.pop(g)
                    for pt2 in range(2):
                        rec = rec_pool.tile([128, HG, 1], F32, tag="rec",
                                            name=f"rec_{s}_{g}_{pt2}")
                        nc.vector.reciprocal(rec[:, :, :], pav[pt2][:, :, HD:HD + 1])
                        rec_b = bass.AP(
                            tensor=rec.tensor,
                            offset=rec.offset,
                            ap=[rec.ap[0], rec.ap[1], [0, HD]],
                        )
                        nc.vector.tensor_mul(
                            self.os[pt2][:, g * HG * HD:(g + 1) * HG * HD]
                            .rearrange("p (h d) -> p h d", d=HD),
                            pav[pt2][:, :, 0:HD],
                            rec_b,
                        )

                def store(self):
                    for pt2 in range(2):
                        r0 = (self.s * 2 + pt2) * 128
                        nc.sync.dma_start(
                            out=out[r0:r0 + 128, :], in_=self.os[pt2][:, :]
                        )

            def attn_slots(s, tail=False):
                """Per-sequence attention as (is_dots, closure) slots in
                pipelined stage order: av runs one pair behind its dots+exp.
                is_dots marks slots after which the PE will wait on ScalarE's
                exp (PSUM slot recycling), i.e. where filler work belongs."""
                sa = SeqAttn(s)
                NP = H // 2  # 6 pairs
                slots = [(True, lambda sa=sa, j=0: sa.dots_exp(j, tail))]
                for j in range(1, NP):
                    slots.append((True, lambda sa=sa, j=j: sa.dots_exp(j, tail)))
                    slots.append((False, lambda sa=sa, j=j - 1: sa.av(j)))
                    if j - 1 == HG // 2 - 1:
                        slots.append((False, lambda sa=sa: sa.norm(0)))
                slots.append((False, lambda sa=sa, j=NP - 1: sa.av(j)))
                slots.append((False, lambda sa=sa: (sa.norm(1), sa.store())))
                return slots

            # ---- prologue ----
            xn0 = emit_xt_load(0)
            wq_nat = emit_w_load("q", split=True)
            # dependency-free warm-up matmuls: the PE would otherwise idle
            # until the first x/W DMA lands (~14us), and the HAM clock gate
            # needs ~3.4us of sustained activity to lift the PE from 1.2 to
            # 2.4 GHz. Warming during the DMA wait makes the real prologue
            # transposes/projections run at full clock.
            warm = const.tile([128, 512], BF16)
            nc.vector.memset(warm[:, :], 0.0)
            wps = ps_proj.tile([128, 512], F32, tag="ps", name="warm_ps")
            for _ in range(16):
                nc.tensor.matmul(
                    wps[:, :], lhsT=warm[:, 0:128], rhs=warm[:, :],
                    start=True, stop=True,
                )
            make_identity(nc, ident[:, :])
            emit_bias_loads()
            for ct in range(CT):
                xt_piece(0, xn0, ct)
            for ct in range(CT):
                for half in range(2):
                    wt_piece("q", wq_nat, ct, half)
            for et in range(ET):
                qk_piece(0, "q", et)
            wk_nat = emit_w_load("k", split=True)
            for ct in range(CT):
                for half in range(2):
                    wt_piece("k", wk_nat, ct, half)
            for et in range(ET):
                qk_piece(0, "k", et)
            wv_nat = emit_w_load("v", split=True)
            for ct in range(CT):
                for half in range(2):
                    wt_piece("v", wv_nat, ct, half)
            for pt in range(4):
                v_piece(0, pt)

            # ---- steady state: interleave attn(c) with chunk c+1's work ----
            for c in range(NPIPE):
                bq_pieces = []
                if c + 1 < NPIPE:
                    xn = emit_xt_load(c + 1)
                    bq_pieces += [
                        (lambda cc=c + 1, xb=xn, ct=ct: xt_piece(cc, xb, ct))
                        for ct in range(CT)
                    ]
                    for et in range(ET):
                        bq_pieces.append(lambda cc=c + 1, et=et: qk_piece(cc, "q", et))
                        bq_pieces.append(lambda cc=c + 1, et=et: qk_piece(cc, "k", et))
                    bq_pieces += [
                        (lambda cc=c + 1, pt=pt: v_piece(cc, pt))
                        for pt in range((c + 1) * 4, (c + 2) * 4)
                    ]
                a_slots = []
                for sloc in range(NCHUNK // P_):
                    a_slots += attn_slots(c * (NCHUNK // P_) + sloc,
                                          tail=(c + 1 == NPIPE))
                # round-robin merge: spread B pieces evenly between A slots
                nb, na = len(bq_pieces), len(a_slots)
                bi = 0
                for ai, (is_dots, slot) in enumerate(a_slots):
                    slot()
                    # keep the even spread, but guarantee one filler right
                    # after every dots+exp slot where the PE stalls next
                    want = ((ai + 1) * nb) // na
                    if is_dots:
                        want = max(want, bi + 1)
                    want = min(want, nb)
                    while bi < want:
                        bq_pieces[bi]()
                        bi += 1
                while bi < nb:
                    bq_pieces[bi]()
                    bi += 1

    nc.finalize()
    return nc


_NC_CACHE = {}


def _get_nc():
    if "nc" not in _NC_CACHE:
        _NC_CACHE["nc"] = build_nc()
    return _NC_CACHE["nc"]


def kernel(x, Wq, bq, Wk, bk, Wv, bv):
    x = np.ascontiguousarray(np.asarray(x, dtype=np.float32))
    args = {
        "Wq": np.ascontiguousarray(np.asarray(Wq, dtype=np.float32)),
        "Wk": np.ascontiguousarray(np.asarray(Wk, dtype=np.float32)),
        "Wv": np.ascontiguousarray(np.asarray(Wv, dtype=np.float32)),
        "bq": np.ascontiguousarray(np.asarray(bq, dtype=np.float32)),
        "bk": np.ascontiguousarray(np.asarray(bk, dtype=np.float32)),
        "bv": np.ascontiguousarray(np.asarray(bv, dtype=np.float32)),
    }
    xf = x.reshape(B * T_ * L * P_, D)
    nc = _get_nc()
    in_maps = [
        {"x": xf[i * NT:(i + 1) * NT], **args} for i in range(N_CORES)
    ]
    res = run_bass_kernel_spmd(nc, in_maps, list(range(N_CORES)))
    outs = [res.results[i]["out"] for i in range(N_CORES)]
    full = np.concatenate(outs, axis=0).reshape(B, T_, L, P_, D)
    return full.astype(np.float32)


# revision 55
# speedup vs baseline: 1.0558x; 1.0139x over previous
"""Multi-head attention (QKV projection + softmax attention) on 8 TRN2 NeuronCores.

Reference computation (per full input):
    x: [2, 8, 4, 256, 768] fp32, H=12 heads, head_dim=64
    q = split_heads(x @ Wq.T + bq); k, v likewise
    out = softmax(q k^T / sqrt(64)) v, heads merged back -> [2, 8, 4, 256, 768]

Sharding: data-parallel over the 2*8*4 = 64 independent (b,t,l) sequences,
8 sequences per core; weights replicated.

Per-core kernel design (all matmuls bf16 inputs, fp32 PSUM accumulate):
  - x and W are cast fp32->bf16 *during* the SWDGE DMA straight into SBUF
    (native layout), then transposed on the TensorE (128x128 PE-transposes
    through PSUM, drained by ScalarE copies) into contraction-major XT/WT.
    Keeping the prologue off the xbar DMA-transpose path matters: Tile cycles
    all DMAs through 8 shared semaphore lanes, and a transpose-heavy DMA
    prologue serializes on false cross-DMA lane dependencies.
  - qT, kT computed feature-major ([e, tok]) so the q k^T matmul reads them
    directly; v computed token-major ([tok, e]) so attn @ v reads it directly.
  - v bias is added via a K=1 matmul row (ones lhsT, bv rhs) which is exact
    because softmax rows sum to one; q/k biases via per-partition
    tensor_scalar adds.
  - softmax skips the max-subtraction (logits are ~N(0,1); exp is safe) and
    gets its denominator for free from a ones-column appended to v, so
    normalization is one reciprocal + one broadcast multiply per head group.
  - dots matmuls are K=64; heads are processed in (even, odd) pairs whose
    operands live at partition offsets 0/64, so the two matmuls run
    concurrently in disjoint PE row groups.
  - engines execute their instruction streams in order, so emission order is
    the schedule: attention stages (dots+exp, attn@v one pair behind, then
    normalize) are interleaved piece-by-piece with the next chunk's
    projection/transpose matmuls so the PE never sits behind a ScalarE exp.
"""

import sys

for _p in ("/opt/trn_rl_repo",):
    if _p not in sys.path:
        sys.path.insert(0, _p)

import numpy as np

import concourse.bass as bass
import concourse.tile as tile
from concourse import bacc, mybir
from concourse.bass_utils import run_bass_kernel_spmd
from concourse.masks import make_identity

N_CORES = 8
B, T_, L, P_, D = 2, 8, 4, 256, 768
H = 12
HD = D // H          # 64
NSEQ = (B * T_ * L) // N_CORES   # 8 sequences per core
NT = NSEQ * P_       # 2048 tokens per core
CT = D // 128        # 6 contraction tiles
ET = D // 128        # 6 output-feature tiles
NCHUNK = 512         # tokens per pipeline chunk
NPIPE = NT // NCHUNK # 4 chunks
SCALE = float(HD) ** -0.5
HG = 6               # heads per PSUM attn-output group

F32 = mybir.dt.float32
BF16 = mybir.dt.bfloat16


def build_nc():
    nc = bacc.Bacc()

    x = nc.dram_tensor("x", [NT, D], F32, kind="ExternalInput")
    Wq = nc.dram_tensor("Wq", [D, D], F32, kind="ExternalInput")
    Wk = nc.dram_tensor("Wk", [D, D], F32, kind="ExternalInput")
    Wv = nc.dram_tensor("Wv", [D, D], F32, kind="ExternalInput")
    bq = nc.dram_tensor("bq", [D], F32, kind="ExternalInput")
    bk = nc.dram_tensor("bk", [D], F32, kind="ExternalInput")
    bv = nc.dram_tensor("bv", [D], F32, kind="ExternalInput")
    out = nc.dram_tensor("out", [NT, D], F32, kind="ExternalOutput")

    w_in = {"q": Wq, "k": Wk, "v": Wv}

    with tile.TileContext(nc) as tc:
        with (
            tc.tile_pool(name="const", bufs=1) as const,
            tc.tile_pool(name="big", bufs=1) as big,
            tc.tile_pool(name="attn", bufs=10) as attn_pool,
            tc.tile_pool(name="rec", bufs=8) as rec_pool,
            tc.tile_pool(name="outp", bufs=6) as out_pool,
            tc.tile_pool(name="xn", bufs=2) as xn_pool,
            tc.tile_pool(name="wn", bufs=1) as wn_pool,
            tc.tile_pool(name="ps_proj", bufs=4, space="PSUM") as ps_proj,
            tc.tile_pool(name="ps_d", bufs=1, space="PSUM") as ps_d,
            tc.tile_pool(name="ps_av", bufs=2, space="PSUM") as ps_av,
        ):
            # ---- constants (biases are loaded after the hot-path DMAs) ----
            bqk = const.tile([128, 2, ET], F32)
            bvr = const.tile([1, D], BF16)
            ones = const.tile([1, 128], BF16)
            nc.vector.memset(ones[:, :], 1.0)
            ident = const.tile([128, 128], BF16)

            def emit_bias_loads():
                nc.gpsimd.dma_start(out=bqk[:, 0, :], in_=bq.rearrange("(t p) -> p t", p=128))
                nc.gpsimd.dma_start(out=bqk[:, 1, :], in_=bk.rearrange("(t p) -> p t", p=128))
                nc.gpsimd.dma_start(out=bvr[:, :], in_=bv.rearrange("(o d) -> o d", o=1))

            WT = {}
            for wk in ("q", "k", "v"):
                WT[wk] = big.tile([128, CT, D], BF16, name=f"WT_{wk}")
            XT = big.tile([128, CT, NT], BF16)     # XT[c%128, c//128, tok]

            qT = big.tile([128, ET, NT], BF16)
            kT = big.tile([128, ET, NT], BF16)
            # v_sb[tok%128, tok//128, h, 0:64] = v ; [..., 64] = 1.0
            v_sb = big.tile([128, NT // 128, H, HD + 1], BF16)
            nc.vector.memset(v_sb[:, :, :, HD:HD + 1], 1.0)

            def emit_xt_load(c):
                xbn = xn_pool.tile([128, NCHUNK // 128, D], BF16, tag="xbn",
                                   name=f"xbn_{c}")
                for hh in range(2):
                    ns = slice(c * NCHUNK + hh * (NCHUNK // 2),
                               c * NCHUNK + (hh + 1) * (NCHUNK // 2))
                    nc.gpsimd.dma_start(
                        out=xbn[:, 2 * hh:2 * hh + 2, :],
                        in_=x[ns, :].rearrange("(t p) c -> p t c", p=128),
                    )
                return xbn

            def xt_piece(c, xbn, ct):
                ns = slice(c * NCHUNK, (c + 1) * NCHUNK)
                pst = ps_proj.tile([128, NCHUNK // 128, 128], BF16, tag="ps",
                                   name=f"pst_{c}_{ct}")
                for ptl in range(NCHUNK // 128):
                    nc.tensor.transpose(
                        pst[:, ptl, :],
                        xbn[:, ptl, ct * 128:(ct + 1) * 128],
                        ident[:, :],
                    )
                nc.scalar.copy(out=XT[:, ct, ns], in_=pst[:, :, :])

            def emit_w_load(wk, split=False):
                wbn = wn_pool.tile([128, CT, D], BF16, tag="wbn",
                                   name=f"wbn_{wk}")
                nh = 2 if split else 1
                step = CT // nh
                for h0 in range(nh):
                    rs = slice(h0 * step * 128, (h0 + 1) * step * 128)
                    nc.gpsimd.dma_start(
                        out=wbn[:, h0 * step:(h0 + 1) * step, :],
                        in_=w_in[wk][rs, :].rearrange("(t p) c -> p t c", p=128),
                    )
                return wbn

            def wt_piece(wk, wbn, ct, half):
                tes = range(3 * half, 3 * half + 3)
                pst = ps_proj.tile([128, 3, 128], BF16, tag="ps",
                                   name=f"pwt_{wk}_{ct}_{half}")
                for k, te in enumerate(tes):
                    nc.tensor.transpose(
                        pst[:, k, :],
                        wbn[:, te, ct * 128:(ct + 1) * 128],
                        ident[:, :],
                    )
                nc.scalar.copy(
                    out=WT[wk][:, ct, 384 * half:384 * (half + 1)],
                    in_=pst[:, :, :],
                )

            def qk_piece(c, wk, et):
                ns = slice(c * NCHUNK, (c + 1) * NCHUNK)
                dst, brow = (qT, 0) if wk == "q" else (kT, 1)
                ps = ps_proj.tile([128, NCHUNK], F32, tag="ps", name="ps_qk")
                for ct in range(CT):
                    nc.tensor.matmul(
                        ps[:, :],
                        lhsT=WT[wk][:, ct, et * 128:(et + 1) * 128],
                        rhs=XT[:, ct, ns],
                        start=(ct == 0),
                        stop=(ct == CT - 1),
                    )
                nc.vector.tensor_scalar_add(
                    dst[:, et, ns], ps[:, :], bqk[:, brow, et:et + 1]
                )

            def v_piece(c, pt):
                for e0, ew in ((0, 512), (512, 256)):
                    ps = ps_proj.tile([128, 512], F32, tag="ps", name="ps_v")
                    for ct in range(CT):
                        nc.tensor.matmul(
                            ps[:, :ew],
                            lhsT=XT[:, ct, pt * 128:(pt + 1) * 128],
                            rhs=WT["v"][:, ct, e0:e0 + ew],
                            start=(ct == 0),
                            stop=False,
                        )
                    nc.tensor.matmul(
                        ps[:, :ew],
                        lhsT=ones[:, :],
                        rhs=bvr[:, e0:e0 + ew],
                        start=False,
                        stop=True,
                    )
                    nc.vector.tensor_copy(
                        out=v_sb[:, pt, e0 // HD:(e0 + ew) // HD, 0:HD],
                        in_=ps[:, :ew].rearrange("p (h d) -> p h d", d=HD),
                    )

            # ---- attention, in stages so projections can be interleaved ----
            class SeqAttn:
                def __init__(self, s):
                    self.s = s
                    self.tok0 = s * P_
                    self.os = [
                        out_pool.tile([128, D], F32, tag="os", name=f"os_{s}_{i}")
                        for i in range(2)
                    ]
                    self.ats = {}
                    self.pav = {}

                def dots_exp(self, jp, tail=False):   # stage 1 for head pair jp
                    s, tok0 = self.s, self.tok0
                    et_h = jp  # = (2*jp)//2
                    at = attn_pool.tile([128, 2, 2, 256], BF16, tag="at",
                                        name=f"at_{s}_{jp}")
                    if tail:
                        # final period: no projection work remains, so borrow
                        # the (otherwise idle) 4-deep projection PSUM pool for
                        # per-head dots tiles; with 4 slots in flight the next
                        # pair's dots no longer wait on this pair's exp.
                        for i in range(2):
                            off = i * HD
                            psd = ps_proj.tile([128, 2, 256], F32, tag="ps",
                                               name=f"psdt_{s}_{jp}_{i}")
                            for qt in range(2):
                                nc.tensor.matmul(
                                    psd[:, qt, :],
                                    lhsT=kT[off:off + HD, et_h,
                                            tok0 + qt * 128:tok0 + (qt + 1) * 128],
                                    rhs=qT[off:off + HD, et_h, tok0:tok0 + P_],
                                    start=True,
                                    stop=True,
                                )
                            nc.scalar.activation(
                                out=at[:, i, :, :],
                                in_=psd[:, :, :],
                                func=mybir.ActivationFunctionType.Exp,
                                scale=SCALE,
                            )
                    else:
                        psd = ps_d.tile([128, 2, 2, 256], F32, tag="ps_d",
                                        name=f"psd_{s}_{jp}")
                        for qt in range(2):
                            for i in range(2):
                                off = i * HD
                                nc.tensor.matmul(
                                    psd[:, i, qt, :],
                                    lhsT=kT[off:off + HD, et_h,
                                            tok0 + qt * 128:tok0 + (qt + 1) * 128],
                                    rhs=qT[off:off + HD, et_h, tok0:tok0 + P_],
                                    start=True,
                                    stop=True,
                                )
                        nc.scalar.activation(
                            out=at[:, :, :, :],
                            in_=psd[:, :, :, :],
                            func=mybir.ActivationFunctionType.Exp,
                            scale=SCALE,
                        )
                    self.ats[2 * jp] = at[:, 0]
                    self.ats[2 * jp + 1] = at[:, 1]

                def av(self, jp):             # stage 2 for head pair jp
                    s = self.s
                    g = jp // (HG // 2)
                    if g not in self.pav:
                        self.pav[g] = [
                            ps_av.tile([128, HG, HD + 1], F32, tag="ps_av",
                                       name=f"pav_{s}_{g}_{i}")
                            for i in range(2)
                        ]
                    for i in range(2):
                        h = 2 * jp + i
                        j = h - g * HG
                        at = self.ats.pop(h)
                        for pt2 in range(2):
                            for qt in range(2):
                                nc.tensor.matmul(
                                    self.pav[g][pt2][:, j, :],
                                    lhsT=at[:, qt, pt2 * 128:(pt2 + 1) * 128],
                                    rhs=v_sb[:, s * 2 + qt, h, :],
                                    start=(qt == 0),
                                    stop=(qt == 1),
                                )

                def norm(self, g):            # stage 3 for head group g
                    s = self.s
                    pav = self.pav.pop(g)
                    for pt2 in range(2):
                        rec = rec_pool.tile([128, HG, 1], F32, tag="rec",
                                            name=f"rec_{s}_{g}_{pt2}")
                        nc.vector.reciprocal(rec[:, :, :], pav[pt2][:, :, HD:HD + 1])
                        rec_b = bass.AP(
                            tensor=rec.tensor,
                            offset=rec.offset,
                            ap=[rec.ap[0], rec.ap[1], [0, HD]],
                        )
                        nc.vector.tensor_mul(
                            self.os[pt2][:, g * HG * HD:(g + 1) * HG * HD]
                            .rearrange("p (h d) -> p h d", d=HD),
                            pav[pt2][:, :, 0:HD],
                            rec_b,
                        )

                def store(self):
                    for pt2 in range(2):
                        r0 = (self.s * 2 + pt2) * 128
                        nc.sync.dma_start(
                            out=out[r0:r0 + 128, :], in_=self.os[pt2][:, :]
                        )

            def attn_slots(s, tail=False):
                """Per-sequence attention as (is_dots, closure) slots in
                pipelined stage order: av runs one pair behind its dots+exp.
                is_dots marks slots after which the PE will wait on ScalarE's
                exp (PSUM slot recycling), i.e. where filler work belongs."""
                sa = SeqAttn(s)
                NP = H // 2  # 6 pairs
                slots = [(True, lambda sa=sa, j=0: sa.dots_exp(j, tail))]
                for j in range(1, NP):
                    slots.append((True, lambda sa=sa, j=j: sa.dots_exp(j, tail)))
                    slots.append((False, lambda sa=sa, j=j - 1: sa.av(j)))
                    if j - 1 == HG // 2 - 1:
                        slots.append((False, lambda sa=sa: sa.norm(0)))
                slots.append((False, lambda sa=sa, j=NP - 1: sa.av(j)))
                slots.append((False, lambda sa=sa: (sa.norm(1), sa.store())))
                return slots

            # ---- prologue ----
            xn0 = emit_xt_load(0)
            wq_nat = emit_w_load("q", split=True)
            # dependency-free warm-up matmuls: the PE would otherwise idle
            # until the first x/W DMA lands (~14us), and the HAM clock gate
            # needs ~3.4us of sustained activity to lift the PE from 1.2 to
            # 2.4 GHz. Warming during the DMA wait makes the real prologue
            # transposes/projections run at full clock.
            warm = const.tile([128, 512], BF16)
            nc.vector.memset(warm[:, :], 0.0)

            def warm_burst(k, n=16):
                wps = ps_proj.tile([128, 512], F32, tag="ps",
                                   name=f"warm_ps_{k}")
                for _ in range(n):
                    nc.tensor.matmul(
                        wps[:, :], lhsT=warm[:, 0:128], rhs=warm[:, :],
                        start=True, stop=True,
                    )

            warm_burst(0, 16)
            make_identity(nc, ident[:, :])
            emit_bias_loads()
            for ct in range(CT):
                xt_piece(0, xn0, ct)
                if ct == 2:
                    warm_burst(1, 4)
            warm_burst(2, 4)
            for ct in range(CT):
                for half in range(2):
                    wt_piece("q", wq_nat, ct, half)
            warm_burst(3, 4)
            for et in range(ET):
                qk_piece(0, "q", et)
            wk_nat = emit_w_load("k", split=True)
            for ct in range(CT):
                for half in range(2):
                    wt_piece("k", wk_nat, ct, half)
            for et in range(ET):
                qk_piece(0, "k", et)
            wv_nat = emit_w_load("v", split=True)
            for ct in range(CT):
                for half in range(2):
                    wt_piece("v", wv_nat, ct, half)
            for pt in range(4):
                v_piece(0, pt)

            # ---- steady state: interleave attn(c) with chunk c+1's work ----
            for c in range(NPIPE):
                bq_pieces = []
                if c + 1 < NPIPE:
                    xn = emit_xt_load(c + 1)
                    bq_pieces += [
                        (lambda cc=c + 1, xb=xn, ct=ct: xt_piece(cc, xb, ct))
                        for ct in range(CT)
                    ]
                    for et in range(ET):
                        bq_pieces.append(lambda cc=c + 1, et=et: qk_piece(cc, "q", et))
                        bq_pieces.append(lambda cc=c + 1, et=et: qk_piece(cc, "k", et))
                    bq_pieces += [
                        (lambda cc=c + 1, pt=pt: v_piece(cc, pt))
                        for pt in range((c + 1) * 4, (c + 2) * 4)
                    ]
                a_slots = []
                for sloc in range(NCHUNK // P_):
                    a_slots += attn_slots(c * (NCHUNK // P_) + sloc,
                                          tail=(c + 1 == NPIPE))
                # round-robin merge: spread B pieces evenly between A slots
                nb, na = len(bq_pieces), len(a_slots)
                bi = 0
                for ai, (is_dots, slot) in enumerate(a_slots):
                    slot()
                    # keep the even spread, but guarantee one filler right
                    # after every dots+exp slot where the PE stalls next
                    want = ((ai + 1) * nb) // na
                    if is_dots:
                        want = max(want, bi + 1)
                    want = min(want, nb)
                    while bi < want:
                        bq_pieces[bi]()
                        bi += 1
                while bi < nb:
                    bq_pieces[bi]()
                    bi += 1

    nc.finalize()
    return nc


_NC_CACHE = {}


def _get_nc():
    if "nc" not in _NC_CACHE:
        _NC_CACHE["nc"] = build_nc()
    return _NC_CACHE["nc"]


def kernel(x, Wq, bq, Wk, bk, Wv, bv):
    x = np.ascontiguousarray(np.asarray(x, dtype=np.float32))
    args = {
        "Wq": np.ascontiguousarray(np.asarray(Wq, dtype=np.float32)),
        "Wk": np.ascontiguousarray(np.asarray(Wk, dtype=np.float32)),
        "Wv": np.ascontiguousarray(np.asarray(Wv, dtype=np.float32)),
        "bq": np.ascontiguousarray(np.asarray(bq, dtype=np.float32)),
        "bk": np.ascontiguousarray(np.asarray(bk, dtype=np.float32)),
        "bv": np.ascontiguousarray(np.asarray(bv, dtype=np.float32)),
    }
    xf = x.reshape(B * T_ * L * P_, D)
    nc = _get_nc()
    in_maps = [
        {"x": xf[i * NT:(i + 1) * NT], **args} for i in range(N_CORES)
    ]
    res = run_bass_kernel_spmd(nc, in_maps, list(range(N_CORES)))
    outs = [res.results[i]["out"] for i in range(N_CORES)]
    full = np.concatenate(outs, axis=0).reshape(B, T_, L, P_, D)
    return full.astype(np.float32)
